# revision 5
# baseline (speedup 1.0000x reference)
"""Trainium2 Bass kernel for nn_DegModel (EDSR-style degradation backbone +
per-pixel KPN), distributed over 8 NeuronCores.

Sharding: one core per (batch, image-half): core i -> batch i//2, half i%2.
Each core runs the whole backbone locally on its 64-row half plus a 17-row
recomputed halo, so no collectives are needed. Bottom halves are processed
vertically flipped (host flips z and the dy axis of the conv weights, both
per-core input data), which makes the on-device geometry identical for all
cores. The only cross-core quantity — the global mean of the predicted noise
channel — is reduced on host from per-core partial sums.

Feature maps live in SBUF as [128 partitions, J slots, 130] with partition
p = channel + 64*parity and the odd-row half skewed one slot down:
lower[c, j] = F[c, 2j], upper[c, j] = F[c, 2j-1]. With this skew a 3x3 conv
over an 8-row output block is exactly 6 full K=128 x M=128 float32r matmuls
(2 per kernel column dx) into one [128, 4, 128] PSUM bank: M columns 0:64
produce the even output rows, 64:128 the odd rows.

conv_in runs in ONE matmul pass per block: the host pre-replicates z into
z_exp[54, J, 130] where partition p packs (parity, ci, dy, dx) with the
spatial shifts baked in, so K=54 covers all 27 taps for both row parities.

conv_out (1x1) + softmax + the 21x21 KPN run per output row in
pixel-partition layout; softmax normalization is folded to after the KPN sum
(y = sum(patch * exp) / sum(exp)). The per-tap multiply runs on DVE in bf16
2x mode; the 441-tap reduction is split between DVE (block tensor_reduce)
and ACT (per-row activation accumulate) to balance the tail; the small
per-row scale ops run on GpSimd.
"""

import sys

sys.path.insert(0, "/opt/trn_rl_repo")

import numpy as np

import concourse.bass as bass
import concourse.tile as tile
from concourse import mybir
from concourse.bass_utils import run_bass_kernel_spmd

KSIZE = 21
NF = 64
NB = 8
IN_NC = 3
B, H, W = 4, 512, 512
h = w = 128
NCH = KSIZE * KSIZE + 1  # 442

N_CORES = 8
J = 44    # feature-buffer slots (2 image rows per slot)
X = 130   # 128 cols + 2 zero pad cols
NMID = 2 * NB
NZP = 54  # z_exp partitions: 2 parities x 3 ch x 9 taps

_cache = {}


def _enable_ldw_opt():
    import concourse.bass_utils as _bu
    if getattr(_bu, "_ldw_opt_patched", False):
        return
    _orig = _bu.run_command

    def _patched(cmd, **kw):
        if isinstance(cmd, list):
            cmd = ["--enable-ldw-opt=true" if c == "--enable-ldw-opt=false"
                   else c for c in cmd]
        return _orig(cmd, **kw)

    _bu.run_command = _patched
    _bu._ldw_opt_patched = True


def _legalize_waits(nc):
    """This walrus build rejects >1 sync wait per instruction; move extra
    waits onto same-engine NOPs inserted immediately before (engines are
    in-order, so semantics are preserved)."""
    for fn in nc.m.functions:
        for blk in fn.blocks:
            out, changed = [], False
            for inst in blk.instructions:
                si = inst.sync_info
                if si is not None and len(si.on_wait) > 1:
                    waits = list(si.on_wait)
                    for wt in waits[:-1]:
                        nop = mybir.InstNoOp(
                            name=nc.get_next_instruction_name(),
                            ins=[], outs=[], engine=inst.engine)
                        nop.sync_info = mybir.SyncInfo(on_wait=[wt], on_update=[])
                        out.append(nop)
                        changed = True
                    inst.sync_info = mybir.SyncInfo(
                        on_wait=[waits[-1]], on_update=list(si.on_update))
                out.append(inst)
            if changed:
                blk.instructions = out


def _build_nc(bias2_zero, bout_zero):
    f32 = mybir.dt.float32
    f32r = mybir.dt.float32r
    nc = bass.Bass()

    z_exp = nc.dram_tensor("z_exp", [NZP, 42, X], f32r, kind="ExternalInput")
    w_in_exp = nc.dram_tensor("w_in_exp", [NZP, 128], f32r,
                              kind="ExternalInput")
    wl1_mid = nc.dram_tensor("wl1_mid", [NMID, 128, 3, 128], f32r,
                             kind="ExternalInput")
    wl2_mid = nc.dram_tensor("wl2_mid", [NMID, 128, 3, 128], f32r,
                             kind="ExternalInput")
    wout_lo = nc.dram_tensor("wout_lo", [128, NCH], f32r, kind="ExternalInput")
    wout_hi = nc.dram_tensor("wout_hi", [128, NCH], f32r, kind="ExternalInput")
    biases = nc.dram_tensor("biases", [NMID + 1, 128, 1], f32,
                            kind="ExternalInput")
    bout_r = nc.dram_tensor("bout_r", [1, NCH], f32r, kind="ExternalInput")
    ones_r = nc.dram_tensor("ones_r", [1, 128], f32r, kind="ExternalInput")
    # per-(channel,row) expanded KPN patch windows, per-partition contiguous
    # (882B per partition -> one descriptor each, 128 per DMA). bf16 for DVE
    # 2x-mode multiplies.
    bf16 = mybir.dt.bfloat16
    # innermost padded to 444 (three zero taps) so the tap dim folds twice
    xw = nc.dram_tensor("xw", [IN_NC, 64, 128, NCH + 2], bf16,
                        kind="ExternalInput")

    ydev = nc.dram_tensor("ydev", [128, IN_NC, 64], f32, kind="ExternalOutput")
    nsdev = nc.dram_tensor("nsdev", [128, 64], f32, kind="ExternalOutput")

    with tile.TileContext(nc) as tc:
        wpool = tc.alloc_tile_pool(name="w", bufs=1)
        gpool = tc.alloc_tile_pool(name="g", bufs=1)
        wmpool = tc.alloc_tile_pool(name="wmid", bufs=3)
        tpool = tc.alloc_tile_pool(name="rtmp", bufs=3)
        ppool = tc.alloc_tile_pool(name="patch", bufs=4)
        epool = tc.alloc_tile_pool(name="exp", bufs=3)
        spool = tc.alloc_tile_pool(name="small", bufs=4)
        psum = tc.alloc_tile_pool(name="ps", bufs=6, space="PSUM")
        psum_o = tc.alloc_tile_pool(name="pso", bufs=2, space="PSUM")

        zin = wpool.tile([NZP, 42, X], f32r)
        w_in = wpool.tile([NZP, 128], f32r)
        wo_lo = wpool.tile([128, NCH], f32r)
        wo_hi = wpool.tile([128, NCH], f32r)
        bias_t = wpool.tile([128, NMID + 1], f32)
        bo_t = wpool.tile([1, NCH], f32r)
        ones_t = wpool.tile([1, 128], f32r)
        nc.sync.dma_start(out=zin, in_=z_exp[:])
        nc.sync.dma_start(out=w_in, in_=w_in_exp[:])
        nc.sync.dma_start(out=wo_lo, in_=wout_lo[:])
        nc.sync.dma_start(out=wo_hi, in_=wout_hi[:])
        nc.sync.dma_start(out=bias_t,
                          in_=biases[:].rearrange("l p one -> p (l one)"))
        nc.sync.dma_start(out=bo_t, in_=bout_r[:])
        nc.sync.dma_start(out=ones_t, in_=ones_r[:])

        feat = gpool.tile([128, J, X], f32r)
        t1 = gpool.tile([128, J, X], f32r)
        nc.vector.memset(feat[:].bitcast(mybir.dt.float32), 0.0)
        nc.vector.memset(t1[:].bitcast(mybir.dt.float32), 0.0)

        relu = mybir.ActivationFunctionType.Relu
        ident = mybir.ActivationFunctionType.Identity

        def conv(src, dst, l1, l2, bias_col, func, k_halo, residual):
            # output region: shard-local rows 0 .. 63 + k_halo -> slots 1..hi
            hi = (64 + k_halo) // 2 + 1      # top slot of even output rows
            blocks = [(s, min(4, hi - s + 1)) for s in range(1, hi + 1, 4)]
            # weight-major inside groups of 4 blocks: consecutive matmuls
            # share the stationary operand (ldw-opt dedups the LDWEIGHTS
            # streams) and 2 spare PSUM banks cover the group boundary.
            for g0 in range(0, len(blocks), 4):
                grp = blocks[g0:g0 + 4]
                tiles = [psum.tile([128, 4, 128], f32, tag="convps",
                                   name=f"cps_{g0}_{i}")
                         for i in range(len(grp))]
                for wi in range(6):
                    dx, phase = wi % 3, wi // 3
                    wt = (l1 if phase == 0 else l2)[:, dx]
                    for (s0, mc), P in zip(grp, tiles):
                        o = s0 + phase
                        nc.tensor.matmul(
                            P[:, 0:mc], wt,
                            src[0:128, o:o + mc, dx:dx + 128],
                            start=(wi == 0), stop=(wi == 5))
                for (s0, mc), P in zip(grp, tiles):
                    if residual is None:
                        nc.scalar.activation(
                            out=dst[0:64, s0:s0 + mc, 1:129],
                            in_=P[0:64, 0:mc],
                            func=func, bias=bias_col[0:64], scale=1.0)
                        nc.scalar.activation(
                            out=dst[64:128, s0 + 1:s0 + 1 + mc, 1:129],
                            in_=P[64:128, 0:mc],
                            func=func, bias=bias_col[64:128], scale=1.0)
                    else:
                        # evacuate via ACT (bias folded), accumulate the
                        # residual on GpSimd (SBUF-only engine, otherwise
                        # idle) to keep DVE off the critical path
                        tmp = tpool.tile([128, 4, 128], f32, tag="rtmp")
                        nc.scalar.activation(
                            out=tmp[:, 0:mc], in_=P[:, 0:mc], func=ident,
                            bias=0.0, scale=1.0)
                        if not bias2_zero:
                            nc.vector.tensor_scalar(
                                out=tmp[:, 0:mc], in0=tmp[:, 0:mc],
                                scalar1=bias_col, scalar2=None,
                                op0=mybir.AluOpType.add)
                        nc.vector.tensor_add(
                            out=dst[0:64, s0:s0 + mc, 1:129],
                            in0=tmp[0:64, 0:mc],
                            in1=residual[0:64, s0:s0 + mc, 1:129])
                        nc.gpsimd.tensor_add(
                            out=dst[64:128, s0 + 1:s0 + 1 + mc, 1:129],
                            in0=tmp[64:128, 0:mc],
                            in1=residual[64:128, s0 + 1:s0 + 1 + mc, 1:129])

        def conv_in():
            # single-pass conv: all 27 taps pre-replicated in zin partitions
            hi = 41
            blocks = [(s, min(4, hi - s + 1)) for s in range(1, hi + 1, 4)]
            for g0 in range(0, len(blocks), 4):
                grp = blocks[g0:g0 + 4]
                tiles = [psum.tile([128, 4, 128], f32, tag="convps",
                                   name=f"cin_{g0}_{i}")
                         for i in range(len(grp))]
                for (s0, mc), P in zip(grp, tiles):
                    nc.tensor.matmul(
                        P[:, 0:mc], w_in[:],
                        zin[0:NZP, s0:s0 + mc, 0:128],
                        start=True, stop=True)
                for (s0, mc), P in zip(grp, tiles):
                    nc.scalar.activation(
                        out=feat[0:64, s0:s0 + mc, 1:129],
                        in_=P[0:64, 0:mc],
                        func=ident, bias=bias_t[0:64, 0:1], scale=1.0)
                    nc.scalar.activation(
                        out=feat[64:128, s0 + 1:s0 + 1 + mc, 1:129],
                        in_=P[64:128, 0:mc],
                        func=ident, bias=bias_t[64:128, 0:1], scale=1.0)

        conv_in()
        for rb in range(NB):
            la, lb = 2 * rb, 2 * rb + 1
            w1a = wmpool.tile([128, 3, 128], f32r, tag="w1")
            w2a = wmpool.tile([128, 3, 128], f32r, tag="w2")
            nc.sync.dma_start(out=w1a, in_=wl1_mid[la])
            nc.sync.dma_start(out=w2a, in_=wl2_mid[la])
            conv(feat, t1, w1a, w2a,
                 bias_t[:, 1 + la:2 + la], relu, 15 - 2 * rb, None)
            w1b = wmpool.tile([128, 3, 128], f32r, tag="w1")
            w2b = wmpool.tile([128, 3, 128], f32r, tag="w2")
            nc.sync.dma_start(out=w1b, in_=wl1_mid[lb])
            nc.sync.dma_start(out=w2b, in_=wl2_mid[lb])
            conv(t1, feat, w1b, w2b,
                 bias_t[:, 1 + lb:2 + lb], ident, 14 - 2 * rb, feat)

        yacc = spool.tile([128, IN_NC, 64], f32, tag="yacc")
        nsacc = spool.tile([128, 64], f32, tag="nsacc")

        NCH2 = NCH + 2   # 444 (two extra zero-product lanes)
        HCH = NCH2 // 2  # 222
        QCH = HCH // 2   # 111
        bf = bf16
        dma_engines = [nc.sync, nc.scalar, nc.gpsimd]
        # pre-zero the rotating ex buffers' two tail lanes once: exp never
        # writes them, so e[442:444] stays 0 and prod's tail is exactly 0
        for i in range(3):
            tz = epool.tile([128, 8, NCH2], bf, tag="ex", name=f"exz_{i}")
            nc.vector.memset(tz[:, :, NCH:NCH2], 0.0)
        # reduce-unit assignment: (y0_block, ch): 7 of 24 units on ACT
        act_units = {(bq, 2) for bq in range(7)}
        for y0 in range(0, 64, 8):
            bq = y0 // 8
            ex2 = epool.tile([128, 8, NCH2], bf, tag="ex")
            ssum2 = spool.tile([128, 8], f32, tag="ssum")
            for r in range(8):
                yl = y0 + r
                if yl % 2 == 0:
                    slot, wsel = yl // 2 + 1, wo_lo
                else:
                    slot, wsel = (yl + 1) // 2 + 1, wo_hi
                Po = psum_o.tile([128, NCH], f32, tag="pout")
                nc.tensor.matmul(Po, feat[:, slot, 1:129], wsel,
                                 start=True, stop=bout_zero)
                if not bout_zero:
                    nc.tensor.matmul(Po, ones_t, bo_t, start=False, stop=True)
                nc.scalar.activation(out=ex2[:, r, 0:NCH], in_=Po,
                                     func=mybir.ActivationFunctionType.Exp,
                                     scale=1.0, accum_out=ssum2[:, r:r + 1])
            rcp2 = spool.tile([128, 8], f32, tag="rcp")
            nc.vector.reciprocal(out=rcp2, in_=ssum2)
            for c in range(IN_NC):
                # patch tap-dim padded to 444 with three zero taps, so the
                # padded products are exactly 0 and two folds are exact
                patch2 = ppool.tile([128, 8, NCH2], bf, tag="patch")
                src_ap = bass.AP(
                    tensor=xw, offset=(c * 64 + y0) * 128 * NCH2,
                    ap=[[NCH2, 128], [128 * NCH2, 8], [1, NCH2]])
                dma_engines[c].dma_start(out=patch2, in_=src_ap)
                prod2 = epool.tile([128, 8, NCH2], bf, tag="prod")
                fold2 = epool.tile([128, 8, HCH], bf, tag="fold")
                pc2 = spool.tile([128, 8, 1], f32, tag="pc")
                nc.vector.tensor_mul(out=prod2, in0=ex2[:], in1=patch2)
                # pairwise folds halve the reduce length at DVE 2x rate
                nc.vector.tensor_add(out=fold2, in0=prod2[:, :, 0:HCH],
                                     in1=prod2[:, :, HCH:NCH2])
                if (bq, c) in act_units:
                    # per-row reduce on ACT via activation accumulate
                    scr = epool.tile([128, HCH], bf, tag="ascr")
                    for r in range(8):
                        nc.scalar.activation(
                            out=scr, in_=fold2[:, r], func=ident,
                            bias=0.0, scale=1.0,
                            accum_out=pc2[:, r])
                else:
                    fold4 = epool.tile([128, 8, QCH], bf, tag="fold4")
                    nc.vector.tensor_add(out=fold4, in0=fold2[:, :, 0:QCH],
                                         in1=fold2[:, :, QCH:HCH])
                    nc.vector.reduce_sum(out=pc2, in_=fold4,
                                         axis=mybir.AxisListType.X)
                nc.gpsimd.tensor_mul(out=yacc[:, c, y0:y0 + 8],
                                     in0=pc2[:, :, 0], in1=rcp2)
            nc.gpsimd.tensor_mul(out=nsacc[:, y0:y0 + 8],
                                 in0=ex2[:, :, NCH - 1], in1=rcp2)

        nc.sync.dma_start(out=ydev[:], in_=yacc)
        nc.sync.dma_start(out=nsdev[:], in_=nsacc)

        for p in (psum_o, psum, spool, epool, ppool, tpool, wmpool, gpool,
                  wpool):
            p.release()

    _legalize_waits(nc)
    return nc


def _stack_l1l2(Wl):
    # Wl [64o, ic, 3, 3] -> L1, L2 [128, 3, 128]
    ic = Wl.shape[1]
    L1 = np.zeros((128, 3, 128), np.float32)
    L2 = np.zeros((128, 3, 128), np.float32)
    for dx in range(3):
        L1[0:ic, dx, 0:64] = Wl[:, :, 1, dx].T
        L1[64:64 + ic, dx, 0:64] = Wl[:, :, 0, dx].T
        L1[0:ic, dx, 64:128] = Wl[:, :, 0, dx].T
        L2[64:64 + ic, dx, 0:64] = Wl[:, :, 2, dx].T
        L2[0:ic, dx, 64:128] = Wl[:, :, 2, dx].T
        L2[64:64 + ic, dx, 64:128] = Wl[:, :, 1, dx].T
    return L1, L2


def _prep_weights(w_in, w1s, w2s, w_out, flip):
    if flip:
        w_in = w_in[:, :, ::-1, :]
        w1s = w1s[:, :, :, ::-1, :]
        w2s = w2s[:, :, :, ::-1, :]
    # w_in_exp [54, 128]: p = par*27 + ci*9 + dy*3 + dx
    w_in_exp = np.zeros((NZP, 128), np.float32)
    for ci in range(IN_NC):
        for dy in range(3):
            for dx in range(3):
                p = ci * 9 + dy * 3 + dx
                w_in_exp[p, 0:64] = w_in[:, ci, dy, dx]
                w_in_exp[27 + p, 64:128] = w_in[:, ci, dy, dx]
    L1m = np.zeros((NMID, 128, 3, 128), np.float32)
    L2m = np.zeros((NMID, 128, 3, 128), np.float32)
    for rb in range(NB):
        L1m[2 * rb], L2m[2 * rb] = _stack_l1l2(w1s[rb])
        L1m[2 * rb + 1], L2m[2 * rb + 1] = _stack_l1l2(w2s[rb])
    wo = w_out[:, :, 0, 0]  # [442, 64]
    wlo = np.zeros((128, NCH), np.float32)
    whi = np.zeros((128, NCH), np.float32)
    wlo[0:64] = wo.T
    whi[64:128] = wo.T
    return w_in_exp, L1m, L2m, wlo, whi


def _build_z_exp(zl):
    """zl [3, 81, 128] shard-local z rows (already flipped for bottom
    halves). Returns z_exp [54, 42, 130] f32:
    even-set  p = ci*9+dy*3+dx, slot j, col c = zl[ci, 2(j-1)+dy-1, c+dx-1]
    odd-set   27+p,             slot j, col c = zl[ci, 2j+dy-2,     c+dx-1]
    (zero outside bounds; c runs 0..127 at buffer cols 0..127, cols 128/129
    are pad-garbage and must stay zero)."""
    z_exp = np.zeros((NZP, 42, X), np.float32)
    zpad = np.zeros((IN_NC, 84, 130), np.float32)
    zpad[:, 1:82, 1:129] = zl  # row r at index r+1, col c at index c+1
    j = np.arange(42)
    c = np.arange(128)
    for ci in range(IN_NC):
        for dy in range(3):
            for dx in range(3):
                p = ci * 9 + dy * 3 + dx
                # even: row 2(j-1)+dy-1 -> padded index 2j+dy-2
                ridx = 2 * (j - 1) + dy - 1 + 1
                valid = (ridx >= 0) & (ridx < 84)
                rr = np.where(valid, ridx, 0)
                vals = zpad[ci][rr][:, c + dx]  # [42, 128]
                vals[~valid] = 0.0
                z_exp[p, :, 0:128] = vals
                # odd: row 2j+dy-2 -> padded index 2j+dy-1
                ridx = 2 * j + dy - 2 + 1
                valid = (ridx >= 0) & (ridx < 84)
                rr = np.where(valid, ridx, 0)
                vals = zpad[ci][rr][:, c + dx]
                vals[~valid] = 0.0
                z_exp[27 + p, :, 0:128] = vals
    return z_exp


def kernel(x, z, eps, w_in, b_in, w1s, b1s, w2s, b2s, w_out, b_out):
    x = np.ascontiguousarray(np.asarray(x, np.float32))
    z = np.asarray(z, np.float32)
    eps = np.asarray(eps, np.float32)
    w_in = np.asarray(w_in, np.float32)
    b_in = np.asarray(b_in, np.float32)
    w1s = np.asarray(w1s, np.float32)
    b1s = np.asarray(b1s, np.float32)
    w2s = np.asarray(w2s, np.float32)
    b2s = np.asarray(b2s, np.float32)
    w_out = np.asarray(w_out, np.float32)
    b_out = np.asarray(b_out, np.float32)

    bias2_zero = bool(np.all(b2s == 0))
    bout_zero = bool(np.all(b_out == 0))
    _enable_ldw_opt()
    key = (bias2_zero, bout_zero)
    if key not in _cache:
        _cache[key] = _build_nc(bias2_zero, bout_zero)
    nc = _cache[key]

    weights = {}
    for flip in (False, True):
        weights[flip] = _prep_weights(w_in, w1s, w2s, w_out, flip)

    biases = np.zeros((NMID + 1, 128, 1), np.float32)
    biases[0, 0:64, 0] = b_in
    biases[0, 64:128, 0] = b_in
    for rb in range(NB):
        biases[1 + 2 * rb, 0:64, 0] = b1s[rb]
        biases[1 + 2 * rb, 64:128, 0] = b1s[rb]
        biases[2 + 2 * rb, 0:64, 0] = b2s[rb]
        biases[2 + 2 * rb, 64:128, 0] = b2s[rb]
    bout_row = np.ascontiguousarray(b_out.reshape(1, NCH))
    ones_row = np.ones((1, 128), np.float32)

    in_maps = []
    for core in range(N_CORES):
        b, half = core // 2, core % 2
        flip = half == 1
        # shard-local z rows 0..80: top zl[r] = z[b, r]; bottom z flipped
        zl = z[b] if not flip else z[b, :, ::-1]
        z_exp = _build_z_exp(np.ascontiguousarray(zl[:, 0:81]))
        # KPN patch windows, fully expanded per output row:
        # xw[c, yl, x0, t, u] = xp[c, 4*y0(yl) + t, 4*x0 + u] with
        # y0 = yl (top) or 127 - yl (bottom flipped), xp = x padded by 10.
        import ml_dtypes
        xp = np.zeros((IN_NC, H + 2 * 10, W + 2 * 10), dtype=ml_dtypes.bfloat16)
        xp[:, 10:10 + H, 10:10 + W] = x[b]
        y0s = np.arange(64) if not flip else (127 - np.arange(64))
        ridx = (4 * y0s)[:, None] + np.arange(KSIZE)[None, :]   # [64, 21]
        cols = 4 * np.arange(128)[:, None] + np.arange(KSIZE)[None, :]
        sub = xp[:, ridx]                 # [3, 64, 21, 532]
        sub = sub[:, :, :, cols]          # [3, 64, 21, 128, 21]
        xw_arr = np.transpose(sub, (0, 1, 3, 2, 4)).reshape(
            IN_NC, 64, 128, KSIZE * KSIZE)
        pad = np.zeros((IN_NC, 64, 128, 3), dtype=xw_arr.dtype)
        xw_arr = np.ascontiguousarray(
            np.concatenate([xw_arr, pad], axis=3))    # [3, 64, 128, 444]
        w_in_exp, L1m, L2m, wlo, whi = weights[flip]
        in_maps.append({
            "z_exp": z_exp, "w_in_exp": w_in_exp,
            "wl1_mid": L1m, "wl2_mid": L2m,
            "wout_lo": wlo, "wout_hi": whi,
            "biases": biases, "bout_r": bout_row, "ones_r": ones_row,
            "xw": xw_arr,
        })

    trace = bool(globals().get("TRACE", False))
    res = run_bass_kernel_spmd(nc, in_maps, core_ids=list(range(N_CORES)),
                               trace=trace)
    globals()["_last_result"] = res

    out = np.zeros((B, IN_NC, h, w), np.float32)
    for bb in range(B):
        ns_sum = (float(res.results[2 * bb]["nsdev"].sum())
                  + float(res.results[2 * bb + 1]["nsdev"].sum()))
        mean_ns = ns_sum / (h * w)
        for half in range(2):
            ydev = res.results[2 * bb + half]["ydev"]  # [128, 3, 64]
            yt = np.transpose(ydev, (1, 2, 0))         # [3, 64, 128]
            if half == 0:
                out[bb, :, 0:64, :] = yt
            else:
                out[bb, :, 64:128, :] = yt[:, ::-1, :]
        out[bb] += mean_ns * eps[bb]
    return out
